# revision 2
# baseline (speedup 1.0000x reference)
"""Multi-head causal attention + output projection on 8 Trainium2 cores.

Problem: B=4, T=2048, H=16, DQK=DV=64, E=1024, causal mask, fp32.

Sharding: core c -> batch b = c//2, head-group g = c%2 (8 heads each).
Each core computes full causal attention for its 8 heads and a partial
output projection (its heads' rows of W_o). Host sums the two partial
projections per batch and adds b_o.

Device algorithm (transposed layout, per head, bf16 operands):
  scores^T(k,q) = K_h Q_h^T           (d on partitions; pre-transposed on host)
  attn^T = exp(scores^T * 1/8)        (no max-subtraction: scores ~ N(0,1);
                                       split between ACT exp and a DVE
                                       Schraudolph bit-trick exp to balance
                                       engine load)
  causal: structural tile skipping + triangular mask on diagonal tiles (gpsimd)
  ctx'^T(65,q) = [V_h | 1]^T attn^T   (PSUM accumulate over k-tiles;
                                       row 64 = softmax denominators)
  ctx^T = ctx'^T[0:64] * (1/sums)     (recip + PE broadcast + DVE mul)
  out(q,E) = ctx^T.T @ W_o_rows       (lhsT=ctx^T, rhs=W_o natural)
"""

import numpy as np
import ml_dtypes

import concourse.bass as bass
import concourse.mybir as mybir
import concourse.tile as tile
from concourse import bacc
from concourse.bass_utils import run_bass_kernel_spmd

B, T, H, D, E = 4, 2048, 16, 64, 1024
HLOC = 8            # heads per core
NCORES = 8
TQ = 512            # q-block size
TK = 128            # k-tile size
NQB = T // TQ       # 4
NHP = HLOC // 2     # 4 head pairs
NKT = T // TK       # 16 k-tiles total
SCALE = 1.0 / np.sqrt(D)

F32 = mybir.dt.float32
F32R = mybir.dt.float32r
BF16 = mybir.dt.bfloat16
I16 = mybir.dt.int16
MM_DT = BF16

# Schraudolph exp constants (bf16 bit pattern via int16):
#   exp(s*SCALE) ~ bitcast_bf16(int16(s * SCH_A + SCH_B))
LOG2E = 1.4426950408889634
SCH_A = float(np.float32(LOG2E * 128.0 * SCALE))
SCH_B = float(np.float32(127.0 * 128.0 - 5.6))
# route every SCH_DEN-th k-tile's exp to the DVE (engine balance)
SCH_NUM, SCH_DEN = 1, 5


def _build_nc():
    nc = bacc.Bacc("TRN2", target_bir_lowering=False, debug=False,
                   num_devices=NCORES, name="mha")
    qt_d = nc.dram_tensor("qt", [HLOC * D, T], MM_DT, kind="ExternalInput")
    kt_d = nc.dram_tensor("kt", [HLOC * D, T], MM_DT, kind="ExternalInput")
    vo_d = nc.dram_tensor("vo", [NHP, T, 224], MM_DT, kind="ExternalInput")
    wo_d = nc.dram_tensor("wo", [HLOC * D, E], MM_DT, kind="ExternalInput")
    tri_d = nc.dram_tensor("tri", [TK, TK], MM_DT, kind="ExternalInput")
    sel_d = nc.dram_tensor("sel", [128, 128], F32R, kind="ExternalInput")
    rcz_d = nc.dram_tensor("rcz", [128, TQ], F32R, kind="ExternalInput")
    out_d = nc.dram_tensor("out", [T, E], F32, kind="ExternalOutput")

    EXP = mybir.ActivationFunctionType.Exp

    with tile.TileContext(nc) as tc:
        with (
            tc.tile_pool(name="const", bufs=1) as const_pool,
            tc.tile_pool(name="ctxT", bufs=1) as ctxT_pool,
            tc.tile_pool(name="qkt", bufs=1) as qkt_pool,
            tc.tile_pool(name="vsb", bufs=1) as v_pool,
            tc.tile_pool(name="attn", bufs=5) as attn_pool,
            tc.tile_pool(name="outsb", bufs=3) as out_pool,
            tc.tile_pool(name="bcs", bufs=2) as bcs_pool,
            tc.tile_pool(name="bcproj", bufs=2, space="PSUM") as bcproj_pool,
            tc.tile_pool(name="scores", bufs=2, space="PSUM") as scores_pool,
            tc.tile_pool(name="ctxA", bufs=1, space="PSUM") as ctxA_pool,
            tc.tile_pool(name="ctxB", bufs=1, space="PSUM") as ctxB_pool,
        ):
            tri_sb = const_pool.tile([TK, TK], MM_DT)
            nc.sync.dma_start(tri_sb[:], tri_d[:])
            sel_sb = const_pool.tile([128, 128], F32R)
            nc.sync.dma_start(sel_sb[:], sel_d[:])
            rc_sb = const_pool.tile([128, TQ], F32R)
            nc.sync.dma_start(rc_sb[:], rcz_d[:])

            ctxT = ctxT_pool.tile([128, NHP, T], MM_DT)

            # all head-pairs resident in SBUF; load order puts hp=0 first so
            # attention starts after the first chunks arrive
            kt_sbs, qt_sbs, v_sbs_all = [], [], []
            for hp in range(NHP):
                kt_sb = qkt_pool.tile([128, T], MM_DT, tag=f"kt{hp}", name="kt_sb")
                qt_sb = qkt_pool.tile([128, T], MM_DT, tag=f"qt{hp}", name="qt_sb")
                vP = v_pool.tile([128, NKT, 224], MM_DT, tag=f"vP{hp}", name="vP")
                kt_sbs.append(kt_sb)
                qt_sbs.append(qt_sb)
                v_sbs_all.append(vP)
            for hp in range(NHP):
                kt_sb, qt_sb = kt_sbs[hp], qt_sbs[hp]
                vP = v_sbs_all[hp]
                hsl = slice(hp * 128, (hp + 1) * 128)
                nc.sync.dma_start(kt_sb[:, 0:512], kt_d[hsl, 0:512])
                nc.sync.dma_start(qt_sb[:, T - TQ:T], qt_d[hsl, T - TQ:T])
                nc.sync.dma_start(vP[:, 0:4], vo_d[hp].rearrange(
                    "(n p) m -> p n m", p=128)[:, 0:4])
                for ch in range(1, 4):
                    csl = slice(ch * 512, (ch + 1) * 512)
                    nc.sync.dma_start(kt_sb[:, csl], kt_d[hsl, csl])
                for ch in range(NQB - 2, -1, -1):  # remaining qt, qb desc
                    csl = slice(ch * TQ, (ch + 1) * TQ)
                    nc.sync.dma_start(qt_sb[:, csl], qt_d[hsl, csl])
                for ch in range(1, 4):
                    ksl = slice(ch * 4, (ch + 1) * 4)
                    nc.sync.dma_start(vP[:, ksl], vo_d[hp].rearrange(
                        "(n p) m -> p n m", p=128)[:, ksl])
            wo_sb = const_pool.tile([128, 4, E], MM_DT)
            nc.sync.dma_start(wo_sb[:], wo_d.rearrange("(n p) e -> p n e", p=128))

            def emit_proj(qt_):
                ot = out_pool.tile([128, E], F32, tag="ot", name="ot")
                for eb in range(E // 512):
                    pp = bcproj_pool.tile([128, TQ], F32, tag="bcproj", name="pp")
                    for kt_ in range(NHP):
                        nc.tensor.matmul(
                            pp[:, 0:512],
                            lhsT=ctxT[:, kt_, qt_ * 128:(qt_ + 1) * 128],
                            rhs=wo_sb[:, kt_, eb * 512:(eb + 1) * 512],
                            start=(kt_ == 0), stop=(kt_ == NHP - 1),
                        )
                    nc.vector.tensor_copy(ot[:, eb * 512:(eb + 1) * 512],
                                          pp[:, 0:512])
                nc.sync.dma_start(out_d[qt_ * 128:(qt_ + 1) * 128, :], ot[:])

            pending_proj = []
            # Software-pipelined emission: before emitting the previous
            # head-pair's normalize chain (which occupies PE/DVE and blocks
            # the ctx banks), emit the next pair's first WARM QK+exp steps so
            # the exp engines never starve at pair transitions.
            WARM = 4
            prev_norm = [None]

            def emit_qk_exp(qb, hp, kk, nfull):
                q0 = max(kk - nfull, 0) * TK
                kt_sb, qt_sb = kt_sbs[hp], qt_sbs[hp]
                scr = scores_pool.tile([128, 2, TQ], F32, tag="scr", name="scr")
                at = attn_pool.tile([128, 2, TQ], MM_DT, tag="attn", name="attn")
                for head in (0, 1):
                    dr = slice(head * D, head * D + D)
                    nc.tensor.matmul(
                        scr[:, head, q0:TQ],
                        lhsT=kt_sb[dr, kk * TK:(kk + 1) * TK],
                        rhs=qt_sb[dr, qb * TQ + q0:(qb + 1) * TQ],
                        start=True, stop=True,
                    )
                # split exp between ACT (table exp) and DVE (Schraudolph
                # bit-trick): route every SCH_DEN-th tile to the DVE
                if (kk % SCH_DEN) < SCH_NUM:
                    nc.vector.tensor_scalar(
                        at[:, :, q0:TQ].bitcast(I16), scr[:, :, q0:TQ],
                        SCH_A, SCH_B,
                        mybir.AluOpType.mult, mybir.AluOpType.add)
                else:
                    nc.scalar.activation(at[:, :, q0:TQ], scr[:, :, q0:TQ],
                                         EXP, scale=float(SCALE))
                if kk >= nfull:
                    # causal mask on the diagonal tile (gpsimd; SBUF-only)
                    for head in (0, 1):
                        nc.gpsimd.tensor_tensor(
                            at[:, head, q0:q0 + TK],
                            at[:, head, q0:q0 + TK], tri_sb[:],
                            mybir.AluOpType.mult)
                return at, q0

            def emit_pv(hp, kk, nk, at, q0, ctx_ts):
                for head in (0, 1):
                    # PV + sums in one M=128 matmul:
                    # even head: [V|1|0..] -> ctx 0:64, sums row 64
                    # odd head:  [0..|1@32|V] -> sums row 32, ctx 64:128
                    nc.tensor.matmul(
                        ctx_ts[head][:, q0:TQ],
                        lhsT=(v_sbs_all[hp][:, kk, 0:128] if head == 0
                              else v_sbs_all[hp][:, kk, 96:224]),
                        rhs=at[:, head, q0:TQ],
                        start=(kk == 0), stop=(kk == nk - 1),
                    )

            def make_norm(hp, qsl, ctx_ts):
                def _norm():
                    # evacuate raw ctx' to SBUF (frees the PSUM banks for the
                    # next pair's deferred PVs); one merged broadcast matmul
                    # and one merged multiply for both heads
                    craw = bcs_pool.tile([128, TQ], F32, tag="bcs", name="craw")
                    for head in (0, 1):
                        ctx_t = ctx_ts[head]
                        srow = D if head == 0 else 32
                        hsl2 = slice(0, D) if head == 0 else slice(D, 128)
                        with nc.allow_low_precision(reason="f32r recips"):
                            nc.vector.reciprocal(rc_sb[srow:srow + 1],
                                                 ctx_t[srow:srow + 1])
                        nc.vector.tensor_copy(craw[hsl2], ctx_t[hsl2])
                    bc = bcproj_pool.tile([128, TQ], F32, tag="bcproj",
                                          name="bc")
                    nc.tensor.matmul(bc[:], lhsT=sel_sb[:], rhs=rc_sb[:],
                                     start=True, stop=True)
                    nc.vector.tensor_mul(ctxT[:, hp, qsl], craw[:], bc[:])
                return _norm

            for qb in range(NQB - 1, -1, -1):
                nk = (qb + 1) * (TQ // TK)
                nfull = nk - (TQ // TK)
                qsl = slice(qb * TQ, (qb + 1) * TQ)

                for hp in range(NHP):
                    ctx_ts = (ctxA_pool.tile([128, TQ], F32, tag="ctxA",
                                             name="ctxA"),
                              ctxB_pool.tile([128, TQ], F32, tag="ctxB",
                                             name="ctxB"))
                    nw = min(WARM, nk)
                    warm = [emit_qk_exp(qb, hp, kk, nfull) for kk in range(nw)]
                    if prev_norm[0] is not None:
                        prev_norm[0]()
                    pk = min(4, nk - 1)
                    for kk in range(nk):
                        if kk == pk and pending_proj:
                            emit_proj(pending_proj.pop(0))
                        if kk < nw:
                            at, q0 = warm[kk]
                        else:
                            at, q0 = emit_qk_exp(qb, hp, kk, nfull)
                        emit_pv(hp, kk, nk, at, q0, ctx_ts)
                    prev_norm[0] = make_norm(hp, qsl, ctx_ts)

                # queue this q-block's projection; emitted interleaved
                pending_proj.extend(range(qb * (TQ // 128), (qb + 1) * (TQ // 128)))
            if prev_norm[0] is not None:
                prev_norm[0]()
            for qt_ in pending_proj:
                emit_proj(qt_)

    nc.compile()
    return nc


_NC_CACHE = {}


def _get_nc():
    if "nc" not in _NC_CACHE:
        _NC_CACHE["nc"] = _build_nc()
    return _NC_CACHE["nc"]


def to_bf16(x):
    return np.asarray(x, dtype=np.float32).astype(ml_dtypes.bfloat16)


def build_in_maps(Q, K, V, W_o):
    # transposed layout [k partitions, q free]: valid iff k <= q
    tri = np.triu(np.ones((TK, TK), dtype=np.float32))
    sel = np.zeros((128, 128), dtype=np.float32)
    sel[D, 0:D] = 1.0     # head even: broadcast recip row 64 to rows 0:64
    sel[32, D:128] = 1.0  # head odd: broadcast recip row 32 to rows 64:128

    in_maps = []
    for c in range(NCORES):
        b, g = c // 2, c % 2
        hs = slice(g * HLOC * D, (g + 1) * HLOC * D)
        qt = np.ascontiguousarray(Q[b][:, hs].T)          # (512, 2048)
        kt = np.ascontiguousarray(K[b][:, hs].T)
        # packed pair stationary, 224 cols: even head reads cols [0:128]
        # = [V_e|1@64|0..], odd head reads [96:224] = [0..|1@32|0..|V_o]
        vo = np.zeros((NHP, T, 224), dtype=np.float32)
        for hp in range(NHP):
            ve = V[b][:, (g * HLOC + 2 * hp) * D:(g * HLOC + 2 * hp + 1) * D]
            vo_ = V[b][:, (g * HLOC + 2 * hp + 1) * D:(g * HLOC + 2 * hp + 2) * D]
            vo[hp, :, 0:D] = ve
            vo[hp, :, D] = 1.0        # even head sums col -> psum row 64
            vo[hp, :, 128] = 1.0      # odd head local col 32 -> psum row 32
            vo[hp, :, 160:224] = vo_
        wo = np.ascontiguousarray(W_o[hs, :])             # (512, 1024)
        in_maps.append({"qt": to_bf16(qt), "kt": to_bf16(kt),
                        "vo": to_bf16(vo), "wo": to_bf16(wo),
                        "tri": to_bf16(tri),
                        "sel": sel, "rcz": np.zeros((128, TQ), dtype=np.float32)})
    return in_maps


def _kernel_numpy(Q, K, V, mask, W_o, b_o):
    """Reference fallback for non-causal masks (never hit in practice)."""
    out = np.empty((B, T, E), dtype=np.float32)
    for b in range(B):
        q = Q[b].reshape(T, H, D).transpose(1, 0, 2)
        k = K[b].reshape(T, H, D).transpose(1, 0, 2)
        v = V[b].reshape(T, H, D).transpose(1, 0, 2)
        s = np.einsum("hqd,hkd->hqk", q, k) / np.sqrt(D)
        s = np.where(mask[b][None], -np.inf, s)
        a = np.exp(s - s.max(-1, keepdims=True))
        a /= a.sum(-1, keepdims=True)
        ctx = np.einsum("hqk,hkd->hqd", a, v).transpose(1, 0, 2).reshape(T, H * D)
        out[b] = ctx @ W_o + b_o
    return out


_CAUSAL = None


def _is_causal(mask):
    global _CAUSAL
    if _CAUSAL is None:
        _CAUSAL = np.triu(np.ones((T, T), dtype=bool), 1)
    m = np.asarray(mask)
    return m.shape == (B, T, T) and all(np.array_equal(m[b], _CAUSAL) for b in range(B))


def kernel(Q, K, V, mask, W_o, b_o):
    Q = np.asarray(Q, dtype=np.float32)
    K = np.asarray(K, dtype=np.float32)
    V = np.asarray(V, dtype=np.float32)
    W_o = np.asarray(W_o, dtype=np.float32)
    b_o = np.asarray(b_o, dtype=np.float32)

    if not _is_causal(mask):
        return _kernel_numpy(Q, K, V, np.asarray(mask, dtype=bool), W_o, b_o)

    in_maps = build_in_maps(Q, K, V, W_o)

    nc = _get_nc()
    res = run_bass_kernel_spmd(nc, in_maps, core_ids=list(range(NCORES)))
    _NC_CACHE["last_results"] = res

    out = np.empty((B, T, E), dtype=np.float32)
    for b in range(B):
        out[b] = res.results[2 * b]["out"] + res.results[2 * b + 1]["out"]
    out += b_o
    return out


# revision 14
# speedup vs baseline: 1.2184x; 1.2184x over previous
"""Multi-head causal attention + output projection on 8 Trainium2 cores.

Problem: B=4, T=2048, H=16, DQK=DV=64, E=1024, causal mask, fp32.

Sharding: core c -> batch b = c//2, head-group g = c%2 (8 heads each).
Each core computes full causal attention for its 8 heads and a partial
output projection (its heads' rows of W_o). Host sums the two partial
projections per batch and adds b_o.

Device algorithm (transposed layout, per head, bf16 operands):
  scores^T(k,q) = K_h Q_h^T           (d on partitions; pre-transposed on host)
  attn^T = exp(scores^T * 1/8)        (no max-subtraction: scores ~ N(0,1);
                                       split between ACT exp and a DVE
                                       Schraudolph bit-trick exp to balance
                                       engine load)
  causal: structural tile skipping + triangular mask on diagonal tiles (gpsimd)
  ctx'^T(65,q) = [V_h | 1]^T attn^T   (PSUM accumulate over k-tiles;
                                       row 64 = softmax denominators)
  ctx^T = ctx'^T[0:64] * (1/sums)     (recip + PE broadcast + DVE mul)
  out(q,E) = ctx^T.T @ W_o_rows       (lhsT=ctx^T, rhs=W_o natural)
"""

import numpy as np
import ml_dtypes

import concourse.bass as bass
import concourse.mybir as mybir
import concourse.tile as tile
from concourse import bacc
from concourse.bass_utils import run_bass_kernel_spmd

B, T, H, D, E = 4, 2048, 16, 64, 1024
HLOC = 8            # heads per core
NCORES = 8
TQ = 512            # q-block size
TK = 128            # k-tile size
NQB = T // TQ       # 4
NHP = HLOC // 2     # 4 head pairs
NKT = T // TK       # 16 k-tiles total
SCALE = 1.0 / np.sqrt(D)

F32 = mybir.dt.float32
F32R = mybir.dt.float32r
BF16 = mybir.dt.bfloat16
I16 = mybir.dt.int16
MM_DT = BF16

# Schraudolph exp constants (bf16 bit pattern via int16):
#   exp(s*SCALE) ~ bitcast_bf16(int16(s * SCH_A + SCH_B))
LOG2E = 1.4426950408889634
SCH_A = float(np.float32(LOG2E * 128.0 * SCALE))
SCH_B = float(np.float32(127.0 * 128.0 - 5.6))
# route every SCH_DEN-th k-tile's exp to the DVE (engine balance)
SCH_NUM, SCH_DEN = 1, 4


def _build_nc():
    nc = bacc.Bacc("TRN2", target_bir_lowering=False, debug=False,
                   num_devices=NCORES, name="mha")
    qt_d = nc.dram_tensor("qt", [HLOC * D, T], MM_DT, kind="ExternalInput")
    kt_d = nc.dram_tensor("kt", [HLOC * D, T], MM_DT, kind="ExternalInput")
    vo_d = nc.dram_tensor("vo", [NHP, T, 224], MM_DT, kind="ExternalInput")
    wo_d = nc.dram_tensor("wo", [HLOC * D, E], MM_DT, kind="ExternalInput")
    tri_d = nc.dram_tensor("tri", [TK, TK], MM_DT, kind="ExternalInput")
    sel_d = nc.dram_tensor("sel", [128, 128], F32R, kind="ExternalInput")
    rcz_d = nc.dram_tensor("rcz", [128, TQ], F32R, kind="ExternalInput")
    out_d = nc.dram_tensor("out", [T, E], F32, kind="ExternalOutput")

    EXP = mybir.ActivationFunctionType.Exp

    with tile.TileContext(nc) as tc:
        with (
            tc.tile_pool(name="const", bufs=1) as const_pool,
            tc.tile_pool(name="ctxT", bufs=1) as ctxT_pool,
            tc.tile_pool(name="qkt", bufs=1) as qkt_pool,
            tc.tile_pool(name="vsb", bufs=1) as v_pool,
            tc.tile_pool(name="attn", bufs=16) as attn_pool,
            tc.tile_pool(name="outsb", bufs=3) as out_pool,
            tc.tile_pool(name="bcs", bufs=2) as bcs_pool,
            tc.tile_pool(name="bcproj", bufs=2, space="PSUM") as bcproj_pool,
            tc.tile_pool(name="scores", bufs=4, space="PSUM") as scores_pool,
            tc.tile_pool(name="ctxA", bufs=1, space="PSUM") as ctxA_pool,
            tc.tile_pool(name="ctxB", bufs=1, space="PSUM") as ctxB_pool,
        ):
            tri_sb = const_pool.tile([TK, TK], MM_DT)
            nc.sync.dma_start(tri_sb[:], tri_d[:])
            sel_sb = const_pool.tile([128, 128], F32R)
            nc.sync.dma_start(sel_sb[:], sel_d[:])
            rc_sb = const_pool.tile([128, TQ], F32R)
            nc.sync.dma_start(rc_sb[:], rcz_d[:])

            ctxT = ctxT_pool.tile([128, NHP, T], MM_DT)

            # all head-pairs resident in SBUF; load order puts hp=0 first so
            # attention starts after the first chunks arrive
            kt_sbs, qt_sbs, v_sbs_all = [], [], []
            for hp in range(NHP):
                kt_sb = qkt_pool.tile([128, T], MM_DT, tag=f"kt{hp}", name="kt_sb")
                qt_sb = qkt_pool.tile([128, T], MM_DT, tag=f"qt{hp}", name="qt_sb")
                vP = v_pool.tile([128, NKT, 224], MM_DT, tag=f"vP{hp}", name="vP")
                kt_sbs.append(kt_sb)
                qt_sbs.append(qt_sb)
                v_sbs_all.append(vP)
            for hp in range(NHP):
                kt_sb, qt_sb = kt_sbs[hp], qt_sbs[hp]
                vP = v_sbs_all[hp]
                hsl = slice(hp * 128, (hp + 1) * 128)
                nc.sync.dma_start(kt_sb[:, 0:512], kt_d[hsl, 0:512])
                nc.sync.dma_start(qt_sb[:, T - TQ:T], qt_d[hsl, T - TQ:T])
                nc.sync.dma_start(vP[:, 0:4], vo_d[hp].rearrange(
                    "(n p) m -> p n m", p=128)[:, 0:4])
                for ch in range(1, 4):
                    csl = slice(ch * 512, (ch + 1) * 512)
                    nc.sync.dma_start(kt_sb[:, csl], kt_d[hsl, csl])
                for ch in range(NQB - 2, -1, -1):  # remaining qt, qb desc
                    csl = slice(ch * TQ, (ch + 1) * TQ)
                    nc.sync.dma_start(qt_sb[:, csl], qt_d[hsl, csl])
                for ch in range(1, 4):
                    ksl = slice(ch * 4, (ch + 1) * 4)
                    nc.sync.dma_start(vP[:, ksl], vo_d[hp].rearrange(
                        "(n p) m -> p n m", p=128)[:, ksl])
            wo_sb = const_pool.tile([128, 4, E], MM_DT)
            nc.sync.dma_start(wo_sb[:], wo_d.rearrange("(n p) e -> p n e", p=128))

            def emit_proj(qt_):
                ot = out_pool.tile([128, E], F32, tag="ot", name="ot")
                for eb in range(E // 512):
                    pp = bcproj_pool.tile([128, TQ], F32, tag="bcproj", name="pp")
                    for kt_ in range(NHP):
                        nc.tensor.matmul(
                            pp[:, 0:512],
                            lhsT=ctxT[:, kt_, qt_ * 128:(qt_ + 1) * 128],
                            rhs=wo_sb[:, kt_, eb * 512:(eb + 1) * 512],
                            start=(kt_ == 0), stop=(kt_ == NHP - 1),
                        )
                    nc.vector.tensor_copy(ot[:, eb * 512:(eb + 1) * 512],
                                          pp[:, 0:512])
                nc.sync.dma_start(out_d[qt_ * 128:(qt_ + 1) * 128, :], ot[:])

            pending_proj = []
            # Software-pipelined emission: before emitting the previous
            # head-pair's normalize chain (which occupies PE/DVE and blocks
            # the ctx banks), emit the next pair's first WARM QK+exp steps so
            # the exp engines never starve at pair transitions.
            WARM = 16
            prev_norm = [None]

            def emit_qk_exp(qb, hp, kk, nfull):
                q0 = max(kk - nfull, 0) * TK
                kt_sb, qt_sb = kt_sbs[hp], qt_sbs[hp]
                at = attn_pool.tile([128, 2, TQ], MM_DT, tag="attn", name="attn")
                for head in (0, 1):
                    scr = scores_pool.tile([128, TQ], F32, tag="scr",
                                           name="scr")
                    dr = slice(head * D, head * D + D)
                    nc.tensor.matmul(
                        scr[:, q0:TQ],
                        lhsT=kt_sb[dr, kk * TK:(kk + 1) * TK],
                        rhs=qt_sb[dr, qb * TQ + q0:(qb + 1) * TQ],
                        start=True, stop=True,
                    )
                    # split exp between ACT (table exp) and DVE (Schraudolph)
                    if ((2 * kk + head) % SCH_DEN) < SCH_NUM:
                        nc.vector.tensor_scalar(
                            at[:, head, q0:TQ].bitcast(I16), scr[:, q0:TQ],
                            SCH_A, SCH_B,
                            mybir.AluOpType.mult, mybir.AluOpType.add)
                    else:
                        nc.scalar.activation(at[:, head, q0:TQ],
                                             scr[:, q0:TQ],
                                             EXP, scale=float(SCALE))
                    if kk >= nfull:
                        # causal mask on the diagonal tile (bf16 DVE 2x)
                        nc.vector.tensor_mul(
                            at[:, head, q0:q0 + TK],
                            at[:, head, q0:q0 + TK], tri_sb[:])
                return at, q0

            def emit_pv(hp, kk, nk, at, q0, ctx_ts):
                for head in (0, 1):
                    # PV + sums in one M=128 matmul:
                    # even head: [V|1|0..] -> ctx 0:64, sums row 64
                    # odd head:  [0..|1@32|V] -> sums row 32, ctx 64:128
                    nc.tensor.matmul(
                        ctx_ts[head][:, q0:TQ],
                        lhsT=(v_sbs_all[hp][:, kk, 0:128] if head == 0
                              else v_sbs_all[hp][:, kk, 96:224]),
                        rhs=at[:, head, q0:TQ],
                        start=(kk == 0), stop=(kk == nk - 1),
                    )

            def make_norm(hp, qsl, ctx_ts):
                def _norm():
                    # evacuate raw ctx' to SBUF (frees the PSUM banks for the
                    # next pair's deferred PVs); one merged broadcast matmul
                    # and one merged multiply for both heads
                    craw = bcs_pool.tile([128, TQ], F32, tag="bcs", name="craw")
                    for head in (0, 1):
                        ctx_t = ctx_ts[head]
                        srow = D if head == 0 else 32
                        hsl2 = slice(0, D) if head == 0 else slice(D, 128)
                        with nc.allow_low_precision(reason="f32r recips"):
                            nc.vector.reciprocal(rc_sb[srow:srow + 1],
                                                 ctx_t[srow:srow + 1])
                        nc.vector.tensor_copy(craw[hsl2], ctx_t[hsl2])
                    bc = bcproj_pool.tile([128, TQ], F32, tag="bcproj",
                                          name="bc")
                    nc.tensor.matmul(bc[:], lhsT=sel_sb[:], rhs=rc_sb[:],
                                     start=True, stop=True)
                    nc.vector.tensor_mul(ctxT[:, hp, qsl], craw[:], bc[:])
                return _norm

            for qb in range(NQB - 1, -1, -1):
                nk = (qb + 1) * (TQ // TK)
                nfull = nk - (TQ // TK)
                qsl = slice(qb * TQ, (qb + 1) * TQ)

                for hp in range(NHP):
                    ctx_ts = (ctxA_pool.tile([128, TQ], F32, tag="ctxA",
                                             name="ctxA"),
                              ctxB_pool.tile([128, TQ], F32, tag="ctxB",
                                             name="ctxB"))
                    nw = min(WARM, nk)
                    warm = [emit_qk_exp(qb, hp, kk, nfull) for kk in range(nw)]
                    if prev_norm[0] is not None:
                        prev_norm[0]()
                    pk = min(4, nk - 1)
                    for kk in range(nk):
                        if kk == pk and pending_proj:
                            emit_proj(pending_proj.pop(0))
                        if kk < nw:
                            at, q0 = warm[kk]
                        else:
                            at, q0 = emit_qk_exp(qb, hp, kk, nfull)
                        emit_pv(hp, kk, nk, at, q0, ctx_ts)
                    prev_norm[0] = make_norm(hp, qsl, ctx_ts)

                # queue this q-block's projection; emitted interleaved
                pending_proj.extend(range(qb * (TQ // 128), (qb + 1) * (TQ // 128)))
            if prev_norm[0] is not None:
                prev_norm[0]()
            for qt_ in pending_proj:
                emit_proj(qt_)

    nc.compile()
    return nc


_NC_CACHE = {}


def _get_nc():
    if "nc" not in _NC_CACHE:
        _NC_CACHE["nc"] = _build_nc()
    return _NC_CACHE["nc"]


def to_bf16(x):
    return np.asarray(x, dtype=np.float32).astype(ml_dtypes.bfloat16)


def build_in_maps(Q, K, V, W_o):
    # transposed layout [k partitions, q free]: valid iff k <= q
    tri = np.triu(np.ones((TK, TK), dtype=np.float32))
    sel = np.zeros((128, 128), dtype=np.float32)
    sel[D, 0:D] = 1.0     # head even: broadcast recip row 64 to rows 0:64
    sel[32, D:128] = 1.0  # head odd: broadcast recip row 32 to rows 64:128

    in_maps = []
    for c in range(NCORES):
        b, g = c // 2, c % 2
        hs = slice(g * HLOC * D, (g + 1) * HLOC * D)
        qt = np.ascontiguousarray(Q[b][:, hs].T)          # (512, 2048)
        kt = np.ascontiguousarray(K[b][:, hs].T)
        # packed pair stationary, 224 cols: even head reads cols [0:128]
        # = [V_e|1@64|0..], odd head reads [96:224] = [0..|1@32|0..|V_o]
        vo = np.zeros((NHP, T, 224), dtype=np.float32)
        for hp in range(NHP):
            ve = V[b][:, (g * HLOC + 2 * hp) * D:(g * HLOC + 2 * hp + 1) * D]
            vo_ = V[b][:, (g * HLOC + 2 * hp + 1) * D:(g * HLOC + 2 * hp + 2) * D]
            vo[hp, :, 0:D] = ve
            vo[hp, :, D] = 1.0        # even head sums col -> psum row 64
            vo[hp, :, 128] = 1.0      # odd head local col 32 -> psum row 32
            vo[hp, :, 160:224] = vo_
        wo = np.ascontiguousarray(W_o[hs, :])             # (512, 1024)
        in_maps.append({"qt": to_bf16(qt), "kt": to_bf16(kt),
                        "vo": to_bf16(vo), "wo": to_bf16(wo),
                        "tri": to_bf16(tri),
                        "sel": sel, "rcz": np.zeros((128, TQ), dtype=np.float32)})
    return in_maps


def _kernel_numpy(Q, K, V, mask, W_o, b_o):
    """Reference fallback for non-causal masks (never hit in practice)."""
    out = np.empty((B, T, E), dtype=np.float32)
    for b in range(B):
        q = Q[b].reshape(T, H, D).transpose(1, 0, 2)
        k = K[b].reshape(T, H, D).transpose(1, 0, 2)
        v = V[b].reshape(T, H, D).transpose(1, 0, 2)
        s = np.einsum("hqd,hkd->hqk", q, k) / np.sqrt(D)
        s = np.where(mask[b][None], -np.inf, s)
        a = np.exp(s - s.max(-1, keepdims=True))
        a /= a.sum(-1, keepdims=True)
        ctx = np.einsum("hqk,hkd->hqd", a, v).transpose(1, 0, 2).reshape(T, H * D)
        out[b] = ctx @ W_o + b_o
    return out


_CAUSAL = None


def _is_causal(mask):
    global _CAUSAL
    if _CAUSAL is None:
        _CAUSAL = np.triu(np.ones((T, T), dtype=bool), 1)
    m = np.asarray(mask)
    return m.shape == (B, T, T) and all(np.array_equal(m[b], _CAUSAL) for b in range(B))


def kernel(Q, K, V, mask, W_o, b_o):
    Q = np.asarray(Q, dtype=np.float32)
    K = np.asarray(K, dtype=np.float32)
    V = np.asarray(V, dtype=np.float32)
    W_o = np.asarray(W_o, dtype=np.float32)
    b_o = np.asarray(b_o, dtype=np.float32)

    if not _is_causal(mask):
        return _kernel_numpy(Q, K, V, np.asarray(mask, dtype=bool), W_o, b_o)

    in_maps = build_in_maps(Q, K, V, W_o)

    nc = _get_nc()
    res = run_bass_kernel_spmd(nc, in_maps, core_ids=list(range(NCORES)))
    _NC_CACHE["last_results"] = res

    out = np.empty((B, T, E), dtype=np.float32)
    for b in range(B):
        out[b] = res.results[2 * b]["out"] + res.results[2 * b + 1]["out"]
    out += b_o
    return out


# revision 17
# speedup vs baseline: 1.2347x; 1.0133x over previous
"""Multi-head causal attention + output projection on 8 Trainium2 cores.

Problem: B=4, T=2048, H=16, DQK=DV=64, E=1024, causal mask, fp32.

Sharding: core c -> batch b = c//2, head-group g = c%2 (8 heads each).
Each core computes full causal attention for its 8 heads and a partial
output projection (its heads' rows of W_o). Host sums the two partial
projections per batch and adds b_o.

Device algorithm (transposed layout, per head, bf16 operands):
  scores^T(k,q) = K_h Q_h^T           (d on partitions; pre-transposed on host)
  attn^T = exp(scores^T * 1/8)        (no max-subtraction: scores ~ N(0,1);
                                       split between ACT exp and a DVE
                                       Schraudolph bit-trick exp to balance
                                       engine load)
  causal: structural tile skipping + triangular mask on diagonal tiles (gpsimd)
  ctx'^T(65,q) = [V_h | 1]^T attn^T   (PSUM accumulate over k-tiles;
                                       row 64 = softmax denominators)
  ctx^T = ctx'^T[0:64] * (1/sums)     (recip + PE broadcast + DVE mul)
  out(q,E) = ctx^T.T @ W_o_rows       (lhsT=ctx^T, rhs=W_o natural)
"""

import numpy as np
import ml_dtypes

import concourse.bass as bass
import concourse.mybir as mybir
import concourse.tile as tile
from concourse import bacc
from concourse.bass_utils import run_bass_kernel_spmd

B, T, H, D, E = 4, 2048, 16, 64, 1024
HLOC = 8            # heads per core
NCORES = 8
TQ = 512            # q-block size
TK = 128            # k-tile size
NQB = T // TQ       # 4
NHP = HLOC // 2     # 4 head pairs
NKT = T // TK       # 16 k-tiles total
SCALE = 1.0 / np.sqrt(D)

F32 = mybir.dt.float32
F32R = mybir.dt.float32r
BF16 = mybir.dt.bfloat16
I16 = mybir.dt.int16
MM_DT = BF16

# Schraudolph exp constants (bf16 bit pattern via int16):
#   exp(s*SCALE) ~ bitcast_bf16(int16(s * SCH_A + SCH_B))
LOG2E = 1.4426950408889634
SCH_A = float(np.float32(LOG2E * 128.0 * SCALE))
SCH_B = float(np.float32(127.0 * 128.0 - 5.6))
# route every SCH_DEN-th k-tile's exp to the DVE (engine balance)
SCH_NUM, SCH_DEN = 1, 4


def _build_nc():
    nc = bacc.Bacc("TRN2", target_bir_lowering=False, debug=False,
                   num_devices=NCORES, name="mha")
    qt_d = nc.dram_tensor("qt", [HLOC * D, T], MM_DT, kind="ExternalInput")
    kt_d = nc.dram_tensor("kt", [HLOC * D, T], MM_DT, kind="ExternalInput")
    vo_d = nc.dram_tensor("vo", [NHP, T, 224], MM_DT, kind="ExternalInput")
    wo_d = nc.dram_tensor("wo", [HLOC * D, E], MM_DT, kind="ExternalInput")
    tri_d = nc.dram_tensor("tri", [TK, TK], MM_DT, kind="ExternalInput")
    sel_d = nc.dram_tensor("sel", [128, 128], F32R, kind="ExternalInput")
    rcz_d = nc.dram_tensor("rcz", [128, TQ], F32R, kind="ExternalInput")
    out_d = nc.dram_tensor("out", [T, E], F32, kind="ExternalOutput")

    EXP = mybir.ActivationFunctionType.Exp

    with tile.TileContext(nc) as tc:
        with (
            tc.tile_pool(name="const", bufs=1) as const_pool,
            tc.tile_pool(name="ctxT", bufs=1) as ctxT_pool,
            tc.tile_pool(name="qkt", bufs=1) as qkt_pool,
            tc.tile_pool(name="vsb", bufs=1) as v_pool,
            tc.tile_pool(name="attn", bufs=16) as attn_pool,
            tc.tile_pool(name="outsb", bufs=3) as out_pool,
            tc.tile_pool(name="bcs", bufs=2) as bcs_pool,
            tc.tile_pool(name="bcproj", bufs=2, space="PSUM") as bcproj_pool,
            tc.tile_pool(name="scores", bufs=4, space="PSUM") as scores_pool,
            tc.tile_pool(name="ctxA", bufs=1, space="PSUM") as ctxA_pool,
            tc.tile_pool(name="ctxB", bufs=1, space="PSUM") as ctxB_pool,
        ):
            tri_sb = const_pool.tile([TK, TK], MM_DT)
            nc.sync.dma_start(tri_sb[:], tri_d[:])
            sel_sb = const_pool.tile([128, 128], F32R)
            nc.sync.dma_start(sel_sb[:], sel_d[:])
            rc_sb = const_pool.tile([128, TQ], F32R)
            nc.sync.dma_start(rc_sb[:], rcz_d[:])

            ctxT = ctxT_pool.tile([128, NHP, T], MM_DT)

            # all head-pairs resident in SBUF; load order puts hp=0 first so
            # attention starts after the first chunks arrive
            kt_sbs, qt_sbs, v_sbs_all = [], [], []
            for hp in range(NHP):
                kt_sb = qkt_pool.tile([128, T], MM_DT, tag=f"kt{hp}", name="kt_sb")
                qt_sb = qkt_pool.tile([128, T], MM_DT, tag=f"qt{hp}", name="qt_sb")
                vP = v_pool.tile([128, NKT, 224], MM_DT, tag=f"vP{hp}", name="vP")
                kt_sbs.append(kt_sb)
                qt_sbs.append(qt_sb)
                v_sbs_all.append(vP)
            for hp in range(NHP):
                kt_sb, qt_sb = kt_sbs[hp], qt_sbs[hp]
                vP = v_sbs_all[hp]
                hsl = slice(hp * 128, (hp + 1) * 128)
                nc.sync.dma_start(kt_sb[:, 0:512], kt_d[hsl, 0:512])
                nc.sync.dma_start(qt_sb[:, T - TQ:T], qt_d[hsl, T - TQ:T])
                nc.sync.dma_start(vP[:, 0:4], vo_d[hp].rearrange(
                    "(n p) m -> p n m", p=128)[:, 0:4])
                for ch in range(1, 4):
                    csl = slice(ch * 512, (ch + 1) * 512)
                    nc.sync.dma_start(kt_sb[:, csl], kt_d[hsl, csl])
                for ch in range(NQB - 2, -1, -1):  # remaining qt, qb desc
                    csl = slice(ch * TQ, (ch + 1) * TQ)
                    nc.sync.dma_start(qt_sb[:, csl], qt_d[hsl, csl])
                for ch in range(1, 4):
                    ksl = slice(ch * 4, (ch + 1) * 4)
                    nc.sync.dma_start(vP[:, ksl], vo_d[hp].rearrange(
                        "(n p) m -> p n m", p=128)[:, ksl])
            wo_sb = const_pool.tile([128, 4, E], MM_DT)
            nc.sync.dma_start(wo_sb[:], wo_d.rearrange("(n p) e -> p n e", p=128))

            def emit_proj(qt_):
                ot = out_pool.tile([128, E], F32, tag="ot", name="ot")
                for eb in range(E // 512):
                    pp = bcproj_pool.tile([128, TQ], F32, tag="bcproj", name="pp")
                    for kt_ in range(NHP):
                        nc.tensor.matmul(
                            pp[:, 0:512],
                            lhsT=ctxT[:, kt_, qt_ * 128:(qt_ + 1) * 128],
                            rhs=wo_sb[:, kt_, eb * 512:(eb + 1) * 512],
                            start=(kt_ == 0), stop=(kt_ == NHP - 1),
                        )
                    nc.vector.tensor_copy(ot[:, eb * 512:(eb + 1) * 512],
                                          pp[:, 0:512])
                nc.sync.dma_start(out_d[qt_ * 128:(qt_ + 1) * 128, :], ot[:])

            pending_proj = []
            # Software-pipelined emission: before emitting the previous
            # head-pair's normalize chain (which occupies PE/DVE and blocks
            # the ctx banks), emit the next pair's first WARM QK+exp steps so
            # the exp engines never starve at pair transitions.
            WARM = 16
            prev_norm = [None]

            def emit_qk_exp(qb, hp, kk, nfull):
                q0 = max(kk - nfull, 0) * TK
                kt_sb, qt_sb = kt_sbs[hp], qt_sbs[hp]
                at = attn_pool.tile([128, 2, TQ], MM_DT, tag="attn", name="attn")
                for head in (0, 1):
                    scr = scores_pool.tile([128, TQ], F32, tag="scr",
                                           name="scr")
                    dr = slice(head * D, head * D + D)
                    nc.tensor.matmul(
                        scr[:, q0:TQ],
                        lhsT=kt_sb[dr, kk * TK:(kk + 1) * TK],
                        rhs=qt_sb[dr, qb * TQ + q0:(qb + 1) * TQ],
                        start=True, stop=True,
                    )
                    # split exp between ACT (table exp) and DVE (Schraudolph)
                    if ((2 * kk + head) % SCH_DEN) < SCH_NUM:
                        nc.vector.tensor_scalar(
                            at[:, head, q0:TQ].bitcast(I16), scr[:, q0:TQ],
                            SCH_A, SCH_B,
                            mybir.AluOpType.mult, mybir.AluOpType.add)
                    else:
                        nc.scalar.activation(at[:, head, q0:TQ],
                                             scr[:, q0:TQ],
                                             EXP, scale=float(SCALE))
                    if kk >= nfull:
                        # causal mask on the diagonal tile (idle gpsimd)
                        nc.gpsimd.tensor_tensor(
                            at[:, head, q0:q0 + TK],
                            at[:, head, q0:q0 + TK], tri_sb[:],
                            mybir.AluOpType.mult)
                return at, q0

            def emit_pv(hp, kk, nk, at, q0, ctx_ts):
                for head in (0, 1):
                    # PV + sums in one M=128 matmul:
                    # even head: [V|1|0..] -> ctx 0:64, sums row 64
                    # odd head:  [0..|1@32|V] -> sums row 32, ctx 64:128
                    nc.tensor.matmul(
                        ctx_ts[head][:, q0:TQ],
                        lhsT=(v_sbs_all[hp][:, kk, 0:128] if head == 0
                              else v_sbs_all[hp][:, kk, 96:224]),
                        rhs=at[:, head, q0:TQ],
                        start=(kk == 0), stop=(kk == nk - 1),
                    )

            def make_norm(hp, qsl, ctx_ts):
                def _norm():
                    # evacuate raw ctx' to SBUF (frees the PSUM banks for the
                    # next pair's deferred PVs); one merged broadcast matmul
                    # and one merged multiply for both heads
                    craw = bcs_pool.tile([128, TQ], F32, tag="bcs", name="craw")
                    for head in (0, 1):
                        ctx_t = ctx_ts[head]
                        srow = D if head == 0 else 32
                        hsl2 = slice(0, D) if head == 0 else slice(D, 128)
                        with nc.allow_low_precision(reason="f32r recips"):
                            nc.vector.reciprocal(rc_sb[srow:srow + 1],
                                                 ctx_t[srow:srow + 1])
                        nc.vector.tensor_copy(craw[hsl2], ctx_t[hsl2])
                    bc = bcproj_pool.tile([128, TQ], F32, tag="bcproj",
                                          name="bc")
                    nc.tensor.matmul(bc[:], lhsT=sel_sb[:], rhs=rc_sb[:],
                                     start=True, stop=True)
                    nc.vector.tensor_mul(ctxT[:, hp, qsl], craw[:], bc[:])
                return _norm

            for qb in range(NQB - 1, -1, -1):
                nk = (qb + 1) * (TQ // TK)
                nfull = nk - (TQ // TK)
                qsl = slice(qb * TQ, (qb + 1) * TQ)

                for hp in range(NHP):
                    ctx_ts = (ctxA_pool.tile([128, TQ], F32, tag="ctxA",
                                             name="ctxA"),
                              ctxB_pool.tile([128, TQ], F32, tag="ctxB",
                                             name="ctxB"))
                    nw = min(WARM, nk)
                    warm = [emit_qk_exp(qb, hp, kk, nfull) for kk in range(nw)]
                    if prev_norm[0] is not None:
                        prev_norm[0]()
                    pk = min(4, nk - 1)
                    for kk in range(nk):
                        if kk == pk and pending_proj:
                            emit_proj(pending_proj.pop(0))
                        if kk < nw:
                            at, q0 = warm[kk]
                        else:
                            at, q0 = emit_qk_exp(qb, hp, kk, nfull)
                        emit_pv(hp, kk, nk, at, q0, ctx_ts)
                    prev_norm[0] = make_norm(hp, qsl, ctx_ts)

                # queue this q-block's projection; emitted interleaved
                pending_proj.extend(range(qb * (TQ // 128), (qb + 1) * (TQ // 128)))
            if prev_norm[0] is not None:
                prev_norm[0]()
            for qt_ in pending_proj:
                emit_proj(qt_)

    nc.compile()
    return nc


_NC_CACHE = {}


def _get_nc():
    if "nc" not in _NC_CACHE:
        _NC_CACHE["nc"] = _build_nc()
    return _NC_CACHE["nc"]


def to_bf16(x):
    return np.asarray(x, dtype=np.float32).astype(ml_dtypes.bfloat16)


def build_in_maps(Q, K, V, W_o):
    # transposed layout [k partitions, q free]: valid iff k <= q
    tri = np.triu(np.ones((TK, TK), dtype=np.float32))
    sel = np.zeros((128, 128), dtype=np.float32)
    sel[D, 0:D] = 1.0     # head even: broadcast recip row 64 to rows 0:64
    sel[32, D:128] = 1.0  # head odd: broadcast recip row 32 to rows 64:128

    in_maps = []
    for c in range(NCORES):
        b, g = c // 2, c % 2
        hs = slice(g * HLOC * D, (g + 1) * HLOC * D)
        qt = np.ascontiguousarray(Q[b][:, hs].T)          # (512, 2048)
        kt = np.ascontiguousarray(K[b][:, hs].T)
        # packed pair stationary, 224 cols: even head reads cols [0:128]
        # = [V_e|1@64|0..], odd head reads [96:224] = [0..|1@32|0..|V_o]
        vo = np.zeros((NHP, T, 224), dtype=np.float32)
        for hp in range(NHP):
            ve = V[b][:, (g * HLOC + 2 * hp) * D:(g * HLOC + 2 * hp + 1) * D]
            vo_ = V[b][:, (g * HLOC + 2 * hp + 1) * D:(g * HLOC + 2 * hp + 2) * D]
            vo[hp, :, 0:D] = ve
            vo[hp, :, D] = 1.0        # even head sums col -> psum row 64
            vo[hp, :, 128] = 1.0      # odd head local col 32 -> psum row 32
            vo[hp, :, 160:224] = vo_
        wo = np.ascontiguousarray(W_o[hs, :])             # (512, 1024)
        in_maps.append({"qt": to_bf16(qt), "kt": to_bf16(kt),
                        "vo": to_bf16(vo), "wo": to_bf16(wo),
                        "tri": to_bf16(tri),
                        "sel": sel, "rcz": np.zeros((128, TQ), dtype=np.float32)})
    return in_maps


def _kernel_numpy(Q, K, V, mask, W_o, b_o):
    """Reference fallback for non-causal masks (never hit in practice)."""
    out = np.empty((B, T, E), dtype=np.float32)
    for b in range(B):
        q = Q[b].reshape(T, H, D).transpose(1, 0, 2)
        k = K[b].reshape(T, H, D).transpose(1, 0, 2)
        v = V[b].reshape(T, H, D).transpose(1, 0, 2)
        s = np.einsum("hqd,hkd->hqk", q, k) / np.sqrt(D)
        s = np.where(mask[b][None], -np.inf, s)
        a = np.exp(s - s.max(-1, keepdims=True))
        a /= a.sum(-1, keepdims=True)
        ctx = np.einsum("hqk,hkd->hqd", a, v).transpose(1, 0, 2).reshape(T, H * D)
        out[b] = ctx @ W_o + b_o
    return out


_CAUSAL = None


def _is_causal(mask):
    global _CAUSAL
    if _CAUSAL is None:
        _CAUSAL = np.triu(np.ones((T, T), dtype=bool), 1)
    m = np.asarray(mask)
    return m.shape == (B, T, T) and all(np.array_equal(m[b], _CAUSAL) for b in range(B))


def kernel(Q, K, V, mask, W_o, b_o):
    Q = np.asarray(Q, dtype=np.float32)
    K = np.asarray(K, dtype=np.float32)
    V = np.asarray(V, dtype=np.float32)
    W_o = np.asarray(W_o, dtype=np.float32)
    b_o = np.asarray(b_o, dtype=np.float32)

    if not _is_causal(mask):
        return _kernel_numpy(Q, K, V, np.asarray(mask, dtype=bool), W_o, b_o)

    in_maps = build_in_maps(Q, K, V, W_o)

    nc = _get_nc()
    res = run_bass_kernel_spmd(nc, in_maps, core_ids=list(range(NCORES)))
    _NC_CACHE["last_results"] = res

    out = np.empty((B, T, E), dtype=np.float32)
    for b in range(B):
        out[b] = res.results[2 * b]["out"] + res.results[2 * b + 1]["out"]
    out += b_o
    return out


# revision 26
# speedup vs baseline: 1.2495x; 1.0120x over previous
"""Multi-head causal attention + output projection on 8 Trainium2 cores.

Problem: B=4, T=2048, H=16, DQK=DV=64, E=1024, causal mask, fp32.

Sharding: core c -> batch b = c//2, head-group g = c%2 (8 heads each).
Each core computes full causal attention for its 8 heads and a partial
output projection (its heads' rows of W_o). Host sums the two partial
projections per batch and adds b_o.

Device algorithm (transposed layout, per head, bf16 operands):
  scores^T(k,q) = K_h Q_h^T           (d on partitions; pre-transposed on host)
  attn^T = exp(scores^T * 1/8)        (no max-subtraction: scores ~ N(0,1);
                                       split between ACT exp and a DVE
                                       Schraudolph bit-trick exp to balance
                                       engine load)
  causal: structural tile skipping + triangular mask on diagonal tiles (gpsimd)
  ctx'^T(65,q) = [V_h | 1]^T attn^T   (PSUM accumulate over k-tiles;
                                       row 64 = softmax denominators)
  ctx^T = ctx'^T[0:64] * (1/sums)     (recip + PE broadcast + DVE mul)
  out(q,E) = ctx^T.T @ W_o_rows       (lhsT=ctx^T, rhs=W_o natural)
"""

import numpy as np
import ml_dtypes

import concourse.bass as bass
import concourse.mybir as mybir
import concourse.tile as tile
from concourse import bacc
from concourse.bass_utils import run_bass_kernel_spmd

B, T, H, D, E = 4, 2048, 16, 64, 1024
HLOC = 8            # heads per core
NCORES = 8
TQ = 512            # q-block size
TK = 128            # k-tile size
NQB = T // TQ       # 4
NHP = HLOC // 2     # 4 head pairs
NKT = T // TK       # 16 k-tiles total
SCALE = 1.0 / np.sqrt(D)

F32 = mybir.dt.float32
F32R = mybir.dt.float32r
BF16 = mybir.dt.bfloat16
I16 = mybir.dt.int16
MM_DT = BF16

# Schraudolph exp constants (bf16 bit pattern via int16):
#   exp(s*SCALE) ~ bitcast_bf16(int16(s * SCH_A + SCH_B))
LOG2E = 1.4426950408889634
SCH_A = float(np.float32(LOG2E * 128.0 * SCALE))
SCH_B = float(np.float32(127.0 * 128.0 - 5.6))
# route every SCH_DEN-th k-tile's exp to the DVE (engine balance)
SCH_NUM, SCH_DEN = 1, 4


def _build_nc():
    nc = bacc.Bacc("TRN2", target_bir_lowering=False, debug=False,
                   num_devices=NCORES, name="mha")
    qt_d = nc.dram_tensor("qt", [HLOC * D, T], MM_DT, kind="ExternalInput")
    kt_d = nc.dram_tensor("kt", [HLOC * D, T], MM_DT, kind="ExternalInput")
    vo_d = nc.dram_tensor("vo", [NHP, T, 224], MM_DT, kind="ExternalInput")
    wo_d = nc.dram_tensor("wo", [HLOC * D, E], MM_DT, kind="ExternalInput")
    tri_d = nc.dram_tensor("tri", [TK, TK], MM_DT, kind="ExternalInput")
    sel_d = nc.dram_tensor("sel", [128, 128], F32R, kind="ExternalInput")
    rcz_d = nc.dram_tensor("rcz", [128, TQ], F32R, kind="ExternalInput")
    out_d = nc.dram_tensor("out", [T, E], F32, kind="ExternalOutput")

    EXP = mybir.ActivationFunctionType.Exp

    with tile.TileContext(nc) as tc:
        with (
            tc.tile_pool(name="const", bufs=1) as const_pool,
            tc.tile_pool(name="ctxT", bufs=1) as ctxT_pool,
            tc.tile_pool(name="qkt", bufs=1) as qkt_pool,
            tc.tile_pool(name="vsb", bufs=1) as v_pool,
            tc.tile_pool(name="attn", bufs=16) as attn_pool,
            tc.tile_pool(name="outsb", bufs=3) as out_pool,
            tc.tile_pool(name="bcs", bufs=2) as bcs_pool,
            tc.tile_pool(name="bcproj", bufs=2, space="PSUM") as bcproj_pool,
            tc.tile_pool(name="scores", bufs=4, space="PSUM") as scores_pool,
            tc.tile_pool(name="ctxA", bufs=1, space="PSUM") as ctxA_pool,
            tc.tile_pool(name="ctxB", bufs=1, space="PSUM") as ctxB_pool,
        ):
            tri_sb = const_pool.tile([TK, TK], MM_DT)
            nc.sync.dma_start(tri_sb[:], tri_d[:])
            sel_sb = const_pool.tile([128, 128], F32R)
            nc.sync.dma_start(sel_sb[:], sel_d[:])
            rc_sb = const_pool.tile([128, TQ], F32R)
            nc.sync.dma_start(rc_sb[:], rcz_d[:])

            ctxT = ctxT_pool.tile([128, NHP, T], MM_DT)

            # all head-pairs resident in SBUF; load order puts hp=0 first so
            # attention starts after the first chunks arrive
            kt_sbs, qt_sbs, v_sbs_all = [], [], []
            for hp in range(NHP):
                kt_sb = qkt_pool.tile([128, T], MM_DT, tag=f"kt{hp}", name="kt_sb")
                qt_sb = qkt_pool.tile([128, T], MM_DT, tag=f"qt{hp}", name="qt_sb")
                vP = v_pool.tile([128, NKT, 224], MM_DT, tag=f"vP{hp}", name="vP")
                kt_sbs.append(kt_sb)
                qt_sbs.append(qt_sb)
                v_sbs_all.append(vP)
            for hp in range(NHP):
                kt_sb, qt_sb = kt_sbs[hp], qt_sbs[hp]
                vP = v_sbs_all[hp]
                hsl = slice(hp * 128, (hp + 1) * 128)
                nc.sync.dma_start(kt_sb[:, 0:512], kt_d[hsl, 0:512])
                nc.sync.dma_start(qt_sb[:, T - TQ:T], qt_d[hsl, T - TQ:T])
                nc.sync.dma_start(vP[:, 0:4], vo_d[hp].rearrange(
                    "(n p) m -> p n m", p=128)[:, 0:4])
                for ch in range(1, 4):
                    csl = slice(ch * 512, (ch + 1) * 512)
                    nc.sync.dma_start(kt_sb[:, csl], kt_d[hsl, csl])
                for ch in range(NQB - 2, -1, -1):  # remaining qt, qb desc
                    csl = slice(ch * TQ, (ch + 1) * TQ)
                    nc.sync.dma_start(qt_sb[:, csl], qt_d[hsl, csl])
                for ch in range(1, 4):
                    ksl = slice(ch * 4, (ch + 1) * 4)
                    nc.sync.dma_start(vP[:, ksl], vo_d[hp].rearrange(
                        "(n p) m -> p n m", p=128)[:, ksl])
            wo_sb = const_pool.tile([128, 4, E], MM_DT)
            nc.sync.dma_start(wo_sb[:], wo_d.rearrange("(n p) e -> p n e", p=128))

            def emit_proj(qt_):
                ot = out_pool.tile([128, E], F32, tag="ot", name="ot")
                for eb in range(E // 512):
                    pp = bcproj_pool.tile([128, TQ], F32, tag="bcproj", name="pp")
                    for kt_ in range(NHP):
                        nc.tensor.matmul(
                            pp[:, 0:512],
                            lhsT=ctxT[:, kt_, qt_ * 128:(qt_ + 1) * 128],
                            rhs=wo_sb[:, kt_, eb * 512:(eb + 1) * 512],
                            start=(kt_ == 0), stop=(kt_ == NHP - 1),
                        )
                    (nc.scalar.copy if eb == 0 else nc.vector.tensor_copy)(
                        ot[:, eb * 512:(eb + 1) * 512], pp[:, 0:512])
                nc.sync.dma_start(out_d[qt_ * 128:(qt_ + 1) * 128, :], ot[:])

            pending_proj = []
            # Software-pipelined emission: before emitting the previous
            # head-pair's normalize chain (which occupies PE/DVE and blocks
            # the ctx banks), emit the next pair's first WARM QK+exp steps so
            # the exp engines never starve at pair transitions.
            WARM = 16
            prev_norm = [None]

            def emit_qk_exp(qb, hp, kk, nfull):
                q0 = max(kk - nfull, 0) * TK
                kt_sb, qt_sb = kt_sbs[hp], qt_sbs[hp]
                at = attn_pool.tile([128, 2, TQ], MM_DT, tag="attn", name="attn")
                for head in (0, 1):
                    scr = scores_pool.tile([128, TQ], F32, tag="scr",
                                           name="scr")
                    dr = slice(head * D, head * D + D)
                    nc.tensor.matmul(
                        scr[:, q0:TQ],
                        lhsT=kt_sb[dr, kk * TK:(kk + 1) * TK],
                        rhs=qt_sb[dr, qb * TQ + q0:(qb + 1) * TQ],
                        start=True, stop=True,
                    )
                    # split exp between ACT (table exp) and DVE (Schraudolph)
                    if ((2 * kk + head) % SCH_DEN) < SCH_NUM:
                        nc.vector.tensor_scalar(
                            at[:, head, q0:TQ].bitcast(I16), scr[:, q0:TQ],
                            SCH_A, SCH_B,
                            mybir.AluOpType.mult, mybir.AluOpType.add)
                    else:
                        nc.scalar.activation(at[:, head, q0:TQ],
                                             scr[:, q0:TQ],
                                             EXP, scale=float(SCALE))
                    if kk >= nfull:
                        # causal mask on the diagonal tile (idle gpsimd)
                        nc.gpsimd.tensor_tensor(
                            at[:, head, q0:q0 + TK],
                            at[:, head, q0:q0 + TK], tri_sb[:],
                            mybir.AluOpType.mult)
                return at, q0

            def emit_pv(hp, kk, nk, at, q0, ctx_ts):
                for head in (0, 1):
                    # PV + sums in one M=128 matmul:
                    # even head: [V|1|0..] -> ctx 0:64, sums row 64
                    # odd head:  [0..|1@32|V] -> sums row 32, ctx 64:128
                    nc.tensor.matmul(
                        ctx_ts[head][:, q0:TQ],
                        lhsT=(v_sbs_all[hp][:, kk, 0:128] if head == 0
                              else v_sbs_all[hp][:, kk, 96:224]),
                        rhs=at[:, head, q0:TQ],
                        start=(kk == 0), stop=(kk == nk - 1),
                    )

            def make_norm(hp, qsl, ctx_ts):
                def _norm():
                    # evacuate raw ctx' to SBUF (frees the PSUM banks for the
                    # next pair's deferred PVs); one merged broadcast matmul
                    # and one merged multiply for both heads
                    craw = bcs_pool.tile([128, TQ], F32, tag="bcs", name="craw")
                    for head in (0, 1):
                        ctx_t = ctx_ts[head]
                        srow = D if head == 0 else 32
                        hsl2 = slice(0, D) if head == 0 else slice(D, 128)
                        with nc.allow_low_precision(reason="f32r recips"):
                            nc.vector.reciprocal(rc_sb[srow:srow + 1],
                                                 ctx_t[srow:srow + 1])
                        nc.vector.tensor_copy(craw[hsl2], ctx_t[hsl2])
                    bc = bcproj_pool.tile([128, TQ], F32, tag="bcproj",
                                          name="bc")
                    nc.tensor.matmul(bc[:], lhsT=sel_sb[:], rhs=rc_sb[:],
                                     start=True, stop=True)
                    nc.vector.tensor_mul(ctxT[:, hp, qsl], craw[:], bc[:])
                return _norm

            for qb in range(NQB - 1, -1, -1):
                nk = (qb + 1) * (TQ // TK)
                nfull = nk - (TQ // TK)
                qsl = slice(qb * TQ, (qb + 1) * TQ)

                for hp in range(NHP):
                    ctx_ts = (ctxA_pool.tile([128, TQ], F32, tag="ctxA",
                                             name="ctxA"),
                              ctxB_pool.tile([128, TQ], F32, tag="ctxB",
                                             name="ctxB"))
                    nw = min(WARM, nk)
                    warm = [emit_qk_exp(qb, hp, kk, nfull) for kk in range(nw)]
                    if prev_norm[0] is not None:
                        prev_norm[0]()
                    pk = min(4, nk - 1)
                    for kk in range(nk):
                        if kk == pk and pending_proj:
                            emit_proj(pending_proj.pop(0))
                        if kk < nw:
                            at, q0 = warm[kk]
                        else:
                            at, q0 = emit_qk_exp(qb, hp, kk, nfull)
                        emit_pv(hp, kk, nk, at, q0, ctx_ts)
                    prev_norm[0] = make_norm(hp, qsl, ctx_ts)

                # queue this q-block's projection; emitted interleaved
                pending_proj.extend(range(qb * (TQ // 128), (qb + 1) * (TQ // 128)))
            if prev_norm[0] is not None:
                prev_norm[0]()
            for qt_ in pending_proj:
                emit_proj(qt_)

    nc.compile()
    return nc


_NC_CACHE = {}


def _get_nc():
    if "nc" not in _NC_CACHE:
        _NC_CACHE["nc"] = _build_nc()
    return _NC_CACHE["nc"]


def to_bf16(x):
    return np.asarray(x, dtype=np.float32).astype(ml_dtypes.bfloat16)


def build_in_maps(Q, K, V, W_o):
    # transposed layout [k partitions, q free]: valid iff k <= q
    tri = np.triu(np.ones((TK, TK), dtype=np.float32))
    sel = np.zeros((128, 128), dtype=np.float32)
    sel[D, 0:D] = 1.0     # head even: broadcast recip row 64 to rows 0:64
    sel[32, D:128] = 1.0  # head odd: broadcast recip row 32 to rows 64:128

    in_maps = []
    for c in range(NCORES):
        b, g = c // 2, c % 2
        hs = slice(g * HLOC * D, (g + 1) * HLOC * D)
        qt = np.ascontiguousarray(Q[b][:, hs].T)          # (512, 2048)
        kt = np.ascontiguousarray(K[b][:, hs].T)
        # packed pair stationary, 224 cols: even head reads cols [0:128]
        # = [V_e|1@64|0..], odd head reads [96:224] = [0..|1@32|0..|V_o]
        vo = np.zeros((NHP, T, 224), dtype=np.float32)
        for hp in range(NHP):
            ve = V[b][:, (g * HLOC + 2 * hp) * D:(g * HLOC + 2 * hp + 1) * D]
            vo_ = V[b][:, (g * HLOC + 2 * hp + 1) * D:(g * HLOC + 2 * hp + 2) * D]
            vo[hp, :, 0:D] = ve
            vo[hp, :, D] = 1.0        # even head sums col -> psum row 64
            vo[hp, :, 128] = 1.0      # odd head local col 32 -> psum row 32
            vo[hp, :, 160:224] = vo_
        wo = np.ascontiguousarray(W_o[hs, :])             # (512, 1024)
        in_maps.append({"qt": to_bf16(qt), "kt": to_bf16(kt),
                        "vo": to_bf16(vo), "wo": to_bf16(wo),
                        "tri": to_bf16(tri),
                        "sel": sel, "rcz": np.zeros((128, TQ), dtype=np.float32)})
    return in_maps


def _kernel_numpy(Q, K, V, mask, W_o, b_o):
    """Reference fallback for non-causal masks (never hit in practice)."""
    out = np.empty((B, T, E), dtype=np.float32)
    for b in range(B):
        q = Q[b].reshape(T, H, D).transpose(1, 0, 2)
        k = K[b].reshape(T, H, D).transpose(1, 0, 2)
        v = V[b].reshape(T, H, D).transpose(1, 0, 2)
        s = np.einsum("hqd,hkd->hqk", q, k) / np.sqrt(D)
        s = np.where(mask[b][None], -np.inf, s)
        a = np.exp(s - s.max(-1, keepdims=True))
        a /= a.sum(-1, keepdims=True)
        ctx = np.einsum("hqk,hkd->hqd", a, v).transpose(1, 0, 2).reshape(T, H * D)
        out[b] = ctx @ W_o + b_o
    return out


_CAUSAL = None


def _is_causal(mask):
    global _CAUSAL
    if _CAUSAL is None:
        _CAUSAL = np.triu(np.ones((T, T), dtype=bool), 1)
    m = np.asarray(mask)
    return m.shape == (B, T, T) and all(np.array_equal(m[b], _CAUSAL) for b in range(B))


def kernel(Q, K, V, mask, W_o, b_o):
    Q = np.asarray(Q, dtype=np.float32)
    K = np.asarray(K, dtype=np.float32)
    V = np.asarray(V, dtype=np.float32)
    W_o = np.asarray(W_o, dtype=np.float32)
    b_o = np.asarray(b_o, dtype=np.float32)

    if not _is_causal(mask):
        return _kernel_numpy(Q, K, V, np.asarray(mask, dtype=bool), W_o, b_o)

    in_maps = build_in_maps(Q, K, V, W_o)

    nc = _get_nc()
    res = run_bass_kernel_spmd(nc, in_maps, core_ids=list(range(NCORES)))
    _NC_CACHE["last_results"] = res

    out = np.empty((B, T, E), dtype=np.float32)
    for b in range(B):
        out[b] = res.results[2 * b]["out"] + res.results[2 * b + 1]["out"]
    out += b_o
    return out
